# revision 69
# baseline (speedup 1.0000x reference)
"""CFConv (SchNet continuous-filter conv) Trainium2 kernel.

Math: out[b,i,f] = Mask[b,i] * sum_j W(d_ij)[f] * X[b,j,f], where the filter
W(d) = ssp(W2 @ ssp(W1 @ rbf(d) + b1) + b2) depends only on the scalar
pairwise distance d. Since W is a smooth 1-D function of d, we refit it
(host-side, weights-only preprocessing) onto one 64-gaussian basis:
    W(d) ~= T.T @ rbf'(d) + Winf          (rbf'_k(d) = exp(-10 (d - mu'_k)^2))
which removes both softplus passes and one matmul from the per-pair work.

Device pipeline per core (one batch element per core, 8 cores):
  1. d2 via one K=5 matmul (d2[i,j] = g_i + g_j - 2<R_i,R_j>), relu+sqrt -> d
  2. DMA per 6-atom block: d -> [2, 576] packed rows (halves)
  3. PE K=2 selector matmul broadcasts d into packed [2x64, 576] PSUM
  4. ACT: Square(d - mu') -> Exp(-10 *) -> rbf (bf16)
  5. PE: 4x filter matmuls (T staged 2x) -> one [128, 1152] PSUM tile
  6. filter x X: per block either ACT-cast then bf16 DVE multiply (2x) or
     direct DVE multiply from PSUM (1x) -- balance chosen so ACT and DVE
     finish together; products land in a 3-block bf16 buffer
  7. DVE pairwise tree over j (6 big ops per 24 atoms) -> outT[:, i]
  8. tail: (outT + Winf*sumX) * mask, PE-transpose, DMA out
"""

import numpy as np
import ml_dtypes

BS, N, F, K = 8, 192, 128, 64
GAMMA = 10.0
MU_MIN, MU_MAX, NB = 0.0, 30.0, 64
LOG2 = float(np.log(2.0))

MU1_MAX = 12.0
FIT_MAX = 13.0

BLK = 8                     # query atoms per block (24 blocks)
NBLK = N // BLK
GRP = 3                     # blocks per tree group (24 atoms)
PAIRS_B = BLK * N           # 1536
HALF = PAIRS_B // 2         # 768
# blocks whose PSUM->bf16 cast runs on ACT (11 of 24, ACT/DVE balanced)
PATH2 = frozenset(b for b in range(NBLK) if b % 2 == 1) - {23}

_CACHE = {}


def _fit_table(W1, b1, W2, b2):
    """Refit the distance->filter map onto K gaussians + constant."""
    d = np.linspace(0.0, FIT_MAX, 6500, dtype=np.float64)
    mu0 = np.linspace(MU_MIN, MU_MAX, NB)
    rbf0 = np.exp(-GAMMA * (d[:, None] - mu0[None, :]) ** 2)
    h = np.logaddexp(0.0, rbf0 @ W1 + b1) - LOG2
    Fd = np.logaddexp(0.0, h @ W2 + b2) - LOG2            # [G, F] true filter
    hinf = np.logaddexp(0.0, b1.astype(np.float64)) - LOG2
    winf = np.logaddexp(0.0, hinf @ W2 + b2) - LOG2       # [F] d->inf filter
    mu1 = np.linspace(0.0, MU1_MAX, K)
    Phi = np.exp(-GAMMA * (d[:, None] - mu1[None, :]) ** 2)
    A = Phi.T @ Phi + 1e-9 * np.eye(K)
    T = np.linalg.solve(A, Phi.T @ (Fd - winf[None, :]))  # [K, F]
    resid = float(np.abs(Phi @ T + winf[None, :] - Fd).max())
    return T, winf, mu1, resid


def _build_nc():
    import concourse.bass as bass
    import concourse.bacc as bacc
    import concourse.mybir as mybir
    from concourse.tile import TileContext
    from contextlib import ExitStack

    dt = mybir.dt
    AF = mybir.ActivationFunctionType
    ALU = mybir.AluOpType

    nc = bacc.Bacc("TRN2", target_bir_lowering=False)

    xt_d = nc.declare_dram_parameter("XT", [F, N], dt.bfloat16, isOutput=False)
    ab_d = nc.declare_dram_parameter("AB", [5, 2 * N], dt.float32, isOutput=False)
    mun_d = nc.declare_dram_parameter("MUNEG", [F, 1], dt.float32, isOutput=False)
    tbl_d = nc.declare_dram_parameter("TBL", [F, F], dt.bfloat16, isOutput=False)
    finf_d = nc.declare_dram_parameter("FINF", [F, 1], dt.float32, isOutput=False)
    mrow_d = nc.declare_dram_parameter("MROW", [1, N], dt.float32r, isOutput=False)
    ones_d = nc.declare_dram_parameter("ONES", [1, F], dt.float32r, isOutput=False)
    sel_d = nc.declare_dram_parameter("SEL", [2, F], dt.float32r, isOutput=False)
    iden_d = nc.declare_dram_parameter("IDEN", [F, F], dt.float32, isOutput=False)
    out_d = nc.declare_dram_parameter("out", [N, F], dt.float32, isOutput=True)

    with TileContext(nc) as tc, ExitStack() as top:
        persist = top.enter_context(tc.tile_pool(name="persist", bufs=1))

        xt_sb = persist.tile([F, N], dt.bfloat16)
        xrep_sb = persist.tile([F, PAIRS_B], dt.bfloat16)
        ab_sb = persist.tile([5, 2 * N], dt.float32)
        mun_sb = persist.tile([F, 1], dt.float32)
        tbl_sb = persist.tile([F, F], dt.bfloat16)
        finf_sb = persist.tile([F, 1], dt.float32)
        mrow_sb = persist.tile([1, N], dt.float32r)
        ones_sb = persist.tile([1, F], dt.float32r)
        sel_sb = persist.tile([2, F], dt.float32r)
        iden_sb = persist.tile([F, F], dt.float32)
        outT = persist.tile([F, N], dt.float32)
        sumx = persist.tile([F, 1], dt.float32)
        corr = persist.tile([F, 1], dt.float32)
        outm = persist.tile([F, 256], dt.float32)
        d_rows = [
            persist.tile([2, HALF], dt.float32r, name=f"drow{b}", tag=f"drow{b}")
            for b in range(NBLK)
        ]

        nc.sync.dma_start(ab_sb[:, :], ab_d[:, :])
        nc.scalar.dma_start(xt_sb[:, :], xt_d[:, :])
        # expand X across the atoms of a block with a step-0 reread copy
        nc.vector.tensor_copy(
            xrep_sb[:, :].rearrange("p (a j) -> p a j", j=N),
            xt_sb[:, :].unsqueeze(1).broadcast_to([F, BLK, N]),
        )
        nc.sync.dma_start(mun_sb[:, :], mun_d[:, :])
        nc.scalar.dma_start(tbl_sb[:, :], tbl_d[:, :])
        nc.sync.dma_start(finf_sb[:, :], finf_d[:, :])
        nc.scalar.dma_start(mrow_sb[:, :], mrow_d[:, :])
        nc.sync.dma_start(ones_sb[:, :], ones_d[:, :])
        nc.scalar.dma_start(sel_sb[:, :], sel_d[:, :])
        nc.sync.dma_start(iden_sb[:, :], iden_d[:, :])
        nc.vector.memset(outm[:, N:256], 0.0)

        # corr[f] = Winf[f] * sum_j X[j, f]
        nc.vector.tensor_reduce(
            sumx[:, :], xrep_sb[:, 0:N], axis=mybir.AxisListType.X, op=ALU.add
        )
        nc.vector.tensor_scalar_mul(corr[:, :], sumx[:, :], finf_sb[:, 0:1])

        # ---- distances ----
        # dca/dcb/d_rows live in the persist pool: closing a pool would make
        # the main-loop pools reuse its SBUF and wait on all 32 row DMAs
        dca = persist.tile([96, N], dt.float32r)
        dcb = persist.tile([96, N], dt.float32r)
        with tc.tile_pool(name="dpsum", bufs=1, space="PSUM") as dpsum:
            d2a = dpsum.tile([96, N], dt.float32)
            d2b = dpsum.tile([96, N], dt.float32)
            nc.tensor.matmul(
                d2a[:, :], ab_sb[:, 0:96], ab_sb[:, N : 2 * N], start=True, stop=True
            )
            nc.tensor.matmul(
                d2b[:, :], ab_sb[:, 96:N], ab_sb[:, N : 2 * N], start=True, stop=True
            )
            nc.vector.tensor_scalar_max(dca[:, :], d2a[:, :], 0.0)
            nc.vector.tensor_scalar_max(dcb[:, :], d2b[:, :], 0.0)
            nc.scalar.activation(dca[:, :], dca[:, :], AF.Sqrt)
            nc.scalar.activation(dcb[:, :], dcb[:, :], AF.Sqrt)
            # per-block packed rows [6,192] -> [2,576] (576 = 3*192 so the
            # DMA AP matcher can split the final dim)
            for b in range(NBLK):
                i0 = b * BLK
                src = dca if i0 < 96 else dcb
                r0 = i0 if i0 < 96 else i0 - 96
                nc.sync.dma_start(d_rows[b][0:2, :], src[r0 : r0 + BLK, :])

        # ---- main blocks ----
        with tc.tile_pool(name="bcp", bufs=1, space="PSUM") as bcp, tc.tile_pool(
            name="fp", bufs=2, space="PSUM"
        ) as fp, tc.tile_pool(name="sqp", bufs=3) as sqp, tc.tile_pool(
            name="rbfp", bufs=3
        ) as rbfp, tc.tile_pool(name="castp", bufs=8) as castp, tc.tile_pool(
            name="pp", bufs=2
        ) as pp, tc.tile_pool(name="trp", bufs=2) as trp:
            P = None
            for bi in range(NBLK):
                g = bi % GRP
                if g == 0:
                    P = pp.tile([F, GRP * PAIRS_B], dt.bfloat16, tag="P")
                p0 = bi * PAIRS_B
                bc = bcp.tile([128, HALF], dt.float32, tag="bc")
                # broadcast: <=512-col chunks (f32r moving-dim limit)
                for c0, cw in ((0, 512), (512, HALF - 512)):
                    nc.tensor.matmul(
                        bc[:, c0 : c0 + cw], sel_sb[:, :],
                        d_rows[bi][0:2, c0 : c0 + cw], start=True, stop=True,
                    )
                sq = sqp.tile([128, HALF], dt.float32, tag="sq")
                nc.scalar.activation(
                    sq[:, :], bc[:, :], AF.Square, bias=mun_sb[:, 0:1]
                )
                rbf = rbfp.tile([128, HALF], dt.bfloat16, tag="rbf")
                nc.scalar.activation(rbf[:, :], sq[:, :], AF.Exp, scale=-GAMMA)

                # filter matmuls into one [F, 1152] PSUM tile; each matmul
                # output must stay inside a 512-f32 bank
                ft = fp.tile([F, PAIRS_B], dt.float32, tag="ft")
                for h in range(2):
                    c = h * HALF
                    while c < (h + 1) * HALF:
                        nxt = min((h + 1) * HALF, (c // 512 + 1) * 512)
                        nc.tensor.matmul(
                            ft[:, c:nxt],
                            tbl_sb[64 * h : 64 * h + 64, :],
                            rbf[64 * h : 64 * h + 64, c - h * HALF : nxt - h * HALF],
                            start=True, stop=True,
                        )
                        c = nxt

                ps = P[:, g * PAIRS_B : (g + 1) * PAIRS_B]
                if bi in PATH2:
                    ca = castp.tile([F, PAIRS_B], dt.bfloat16, tag="cast")
                    nc.scalar.activation(ca[:, :], ft[:, :], AF.Copy)
                    nc.vector.tensor_tensor(
                        ps, ca[:, :], xrep_sb[:, :], op=ALU.mult
                    )
                else:
                    nc.vector.tensor_tensor(
                        ps, ft[:, :], xrep_sb[:, :], op=ALU.mult
                    )

                if g == GRP - 1:
                    # tree-reduce the 3-block product buffer over j
                    gi = (bi // GRP) * GRP * BLK     # first atom of group
                    ni = GRP * BLK                   # 24 atoms
                    pv = P[:, :].rearrange("p (i j) -> p i j", j=N)
                    t1 = trp.tile([F, ni * 96], dt.bfloat16, tag="t1")
                    v1 = t1[:, :].rearrange("p (i j) -> p i j", j=96)
                    nc.vector.tensor_tensor(
                        v1, pv[:, :, 0:96], pv[:, :, 96:192], op=ALU.add
                    )
                    t2 = trp.tile([F, ni * 48], dt.bfloat16, tag="t2")
                    v2 = t2[:, :].rearrange("p (i j) -> p i j", j=48)
                    nc.vector.tensor_tensor(
                        v2, v1[:, :, 0:48], v1[:, :, 48:96], op=ALU.add
                    )
                    t3 = trp.tile([F, ni * 24], dt.bfloat16, tag="t3")
                    v3 = t3[:, :].rearrange("p (i j) -> p i j", j=24)
                    nc.vector.tensor_tensor(
                        v3, v2[:, :, 0:24], v2[:, :, 24:48], op=ALU.add
                    )
                    t4 = trp.tile([F, ni * 12], dt.float32, tag="t4")
                    v4 = t4[:, :].rearrange("p (i j) -> p i j", j=12)
                    nc.vector.tensor_tensor(
                        v4, v3[:, :, 0:12], v3[:, :, 12:24], op=ALU.add
                    )
                    t5 = trp.tile([F, ni * 6], dt.float32, tag="t5")
                    v5 = t5[:, :].rearrange("p (i j) -> p i j", j=6)
                    nc.vector.tensor_tensor(
                        v5, v4[:, :, 0:6], v4[:, :, 6:12], op=ALU.add
                    )
                    nc.vector.tensor_reduce(
                        outT[:, gi : gi + ni],
                        t5[:, :].rearrange("p (i j) -> p i j", j=6),
                        axis=mybir.AxisListType.X, op=ALU.add,
                    )

            # ---- tail: correction + mask, transpose, store ----
            mbc = fp.tile([F, N], dt.float32, tag="ft")
            nc.tensor.matmul(
                mbc[:, :], ones_sb[:, :], mrow_sb[:, :], start=True, stop=True
            )
            nc.vector.scalar_tensor_tensor(
                outm[:, 0:N], outT[:, :], corr[:, 0:1], mbc[:, :],
                op0=ALU.add, op1=ALU.mult,
            )
            to1 = fp.tile([128, F], dt.float32, tag="ft")
            to2 = fp.tile([128, F], dt.float32, tag="ft")
            nc.tensor.transpose(to1[:, :], outm[:, 0:128], iden_sb[:, :])
            nc.tensor.transpose(to2[:, :], outm[:, 128:256], iden_sb[:, :])
            o1 = persist.tile([128, F], dt.float32)
            o2 = persist.tile([64, F], dt.float32)
            nc.scalar.activation(o1[:, :], to1[:, :], AF.Copy)
            nc.scalar.activation(o2[:, :], to2[0:64, :], AF.Copy)
            nc.sync.dma_start(out_d[0:128, :], o1[:, :])
            nc.scalar.dma_start(out_d[128:N, :], o2[:, :])

    nc.compile()
    return nc


def _prepare_inputs(X, R, Mask, W1, b1, W2, b2):
    T, winf, mu1, resid = _fit_table(
        np.asarray(W1, np.float64), np.asarray(b1, np.float64),
        np.asarray(W2, np.float64), np.asarray(b2, np.float64),
    )
    bf16 = ml_dtypes.bfloat16
    mun = np.tile(-mu1.astype(np.float32), 2).reshape(F, 1)
    tbl = np.tile(
        np.ascontiguousarray(T.astype(np.float32)).astype(bf16), (2, 1)
    )  # [128, F]: table staged 2x for the packed halves
    finf = winf.astype(np.float32).reshape(F, 1)
    iden = np.eye(F, dtype=np.float32)
    sel = np.zeros((2, F), np.float32)
    sel[0, 0:64] = 1.0
    sel[1, 64:128] = 1.0
    in_maps = []
    for b in range(BS):
        Rs = np.asarray(R[b, 0], np.float32)              # [N, 3]
        g = (Rs * Rs).sum(axis=1).astype(np.float32)      # [N]
        alhs = np.concatenate(
            [Rs.T, g[None, :], np.ones((1, N), np.float32)], axis=0
        )
        arhs = np.concatenate(
            [-2.0 * Rs.T, np.ones((1, N), np.float32), g[None, :]], axis=0
        )
        ab = np.concatenate([alhs, arhs], axis=1)         # [5, 384]
        xt = np.ascontiguousarray(np.asarray(X[b, 0], np.float32).T).astype(bf16)  # [F, N]
        mrow = np.ascontiguousarray(
            np.asarray(Mask[b, 0, :, 0], np.float32).reshape(1, N)
        )
        in_maps.append({
            "XT": xt,
            "AB": np.ascontiguousarray(ab),
            "MUNEG": mun.copy(),
            "TBL": tbl.copy(),
            "FINF": finf.copy(),
            "MROW": mrow,
            "ONES": np.ones((1, F), np.float32),
            "SEL": sel.copy(),
            "IDEN": iden.copy(),
        })
    return in_maps, resid


def kernel(X, R, Mask, W1, b1, W2, b2):
    from concourse.bass_utils import run_bass_kernel_spmd

    in_maps, _resid = _prepare_inputs(X, R, Mask, W1, b1, W2, b2)
    if "nc" not in _CACHE:
        _CACHE["nc"] = _build_nc()
    nc = _CACHE["nc"]
    res = run_bass_kernel_spmd(nc, in_maps, core_ids=list(range(BS)))
    out = np.stack([r["out"] for r in res.results], axis=0)[:, None]  # [8,1,192,128]
    return out.astype(np.float32)
